# revision 29
# baseline (speedup 1.0000x reference)
"""DeepseekV3 MLA attention forward on 8 Trainium2 NeuronCores.

Sharding: core c -> batch c//4, head group c%4 (4 of 16 heads).
Per-core Bass kernel computes the full MLA forward for its (batch, heads)
slice; host sums the 4 partial wo-projections per batch.

All rope interleaving / head packing / softmax scale are folded into
host-side weight preprocessing. Matmuls run in bf16 with f32 PSUM
accumulation. Attention uses S^T = K^T Q chunks (keys on partitions,
queries on free dim), exp without max subtraction (scores are bounded),
multiplicative causal mask tiles, and a ones-augmented V so the softmax
denominator falls out of the PV matmul.
"""

import math

import numpy as np
import ml_dtypes

import concourse.bass as bass
import concourse.tile as tile
import concourse.mybir as mybir
from concourse import bacc
from concourse.bass_utils import run_bass_kernel_spmd

BF16 = mybir.dt.bfloat16
F32 = mybir.dt.float32
F8 = mybir.dt.float8e4
DR = mybir.MatmulPerfMode.DoubleRow
AF = mybir.ActivationFunctionType

# fp8 hi/lo pair scales for stage A (see _f8pair); products carry S_H*S_WA
S_H = 16.0
S_WA = 1024.0

# ---- model config (hardcoded to match the problem spec) ----
HIDDEN = 2048
N_HEADS = 16
Q_LORA = 1536
KV_LORA = 512
NOPE = 128
ROPE = 64
VHD = 128
QHD = NOPE + ROPE  # 192
BASE = 10000.0
SCALE = 40.0
ORIG_MAX = 4096
BETA_FAST = 32
BETA_SLOW = 1
EPS = 1e-6
B = 2
S = 2048

N_CORES = 8
HL = 4          # heads per core
P = 128
TT = S // P     # 16 token tiles
QC = S // 512   # 4 query chunks of 512
KT = S // P     # 16 key tiles

_m = 0.1 * math.log(SCALE) + 1.0
SOFT_SCALE = (QHD ** -0.5) * _m * _m


def _yarn_cos_sin(seq_len):
    dim = ROPE
    ar = np.arange(0, dim, 2, dtype=np.float32)
    freq_extra = 1.0 / BASE ** (ar / dim)
    freq_inter = 1.0 / (SCALE * BASE ** (ar / dim))
    low = math.floor(dim * math.log(ORIG_MAX / (BETA_FAST * 2 * math.pi)) / (2 * math.log(BASE)))
    high = math.ceil(dim * math.log(ORIG_MAX / (BETA_SLOW * 2 * math.pi)) / (2 * math.log(BASE)))
    low, high = max(low, 0), min(high, dim - 1)
    denom = (high - low) if high != low else 0.001
    ramp = np.clip((np.arange(dim // 2, dtype=np.float32) - low) / denom, 0.0, 1.0)
    inv_freq_mask = 1.0 - ramp
    inv_freq = freq_inter * (1.0 - inv_freq_mask) + freq_extra * inv_freq_mask
    t = np.arange(seq_len, dtype=np.float32)
    freqs = np.outer(t, inv_freq)
    emb = np.concatenate([freqs, freqs], axis=-1)
    # mscale ratio is 1.0 for this config
    return np.cos(emb).astype(np.float32), np.sin(emb).astype(np.float32)


_PERM64 = np.concatenate([np.arange(0, 64, 2), np.arange(1, 64, 2)])


def _bf16(x):
    return np.ascontiguousarray(x.astype(ml_dtypes.bfloat16))


def _f8pair(x, s):
    """x*s as e4m3 hi plus e4m3 residual at the SAME scale, stacked on axis 1."""
    xs = np.asarray(x, np.float32) * s
    hi = np.clip(xs, -240, 240).astype(ml_dtypes.float8_e4m3)
    lo = np.clip(xs - hi.astype(np.float32), -240, 240).astype(ml_dtypes.float8_e4m3)
    return np.ascontiguousarray(np.stack([hi, lo], axis=1))


def _emit_stage_c(nc, tc, psS, kn_sb, q_nope, qpe_rot, krotT, vaug, attnT8, masks,
                  ident_sb):
    with (
        tc.tile_pool(name="pt", bufs=14) as ptp,
        tc.tile_pool(name="workC", bufs=4) as workC,
        tc.tile_pool(name="maskp", bufs=1) as maskp,
        tc.tile_pool(name="psO", bufs=3, space="PSUM") as psO,
        tc.tile_pool(name="psTC", bufs=2, space="PSUM") as psTC,
    ):
        mask_sb = maskp.tile([P, 896], BF16, tag="mask_sb")
        nc.sync.dma_start(mask_sb[:], masks[:])
        for qc in range(QC):
            for h in range(HL):
                nkt = 4 * qc + 4
                pts = []
                offs = []
                for kt in range(nkt):
                    diag = (kt // 4 == qc)
                    off = (kt % 4) * P if diag else 0
                    w = 512 - off
                    ss = psS.tile([P, 512], F32, tag="ss")
                    nc.tensor.matmul(
                        ss[:, 0:w], kn_sb[:, h, kt * P:(kt + 1) * P],
                        q_nope[:, h, qc * 512 + off:(qc + 1) * 512],
                        start=True, stop=False)
                    nc.tensor.matmul(
                        ss[:, 0:w], krotT[:, kt * P:(kt + 1) * P],
                        qpe_rot[:, h, qc * 512 + off:(qc + 1) * 512],
                        start=False, stop=True)
                    pt_t = ptp.tile([P, 512], BF16, tag="pt")
                    nc.scalar.activation(pt_t[:, 0:w], ss[:, 0:w], AF.Exp,
                                         scale=SOFT_SCALE)
                    if diag:
                        nc.vector.tensor_mul(pt_t[:, 0:P], pt_t[:, 0:P],
                                             mask_sb[:, 384:384 + P])
                    pts.append(pt_t)
                    offs.append(off)
                for qsub in range(4):
                    qt = 4 * qc + qsub
                    po = psO.tile([P, VHD + 1], F32, tag="po")
                    for kt in range(qt + 1):
                        o = qsub * P - offs[kt]
                        nc.tensor.matmul(
                            po[:], pts[kt][:, o:o + P],
                            vaug[:, h, kt, 0:VHD + 1],
                            start=(kt == 0), stop=(kt == qt))
                    rd = workC.tile([P, 1], F32, tag="rd")
                    nc.vector.reciprocal(rd[:], po[:, VHD:VHD + 1])
                    at = workC.tile([P, VHD], BF16, tag="at")
                    nc.vector.tensor_scalar_mul(at[:], po[:, 0:VHD], rd[:])
                    ptt = psTC.tile([P, P], BF16, tag="ptt")
                    nc.tensor.transpose(ptt[:], at[:], ident_sb[:])
                    nc.vector.tensor_copy(
                        attnT8[:, h, 0, qt * P:(qt + 1) * P], ptt[:])
                    nc.vector.tensor_sub(
                        attnT8[:, h, 1, qt * P:(qt + 1) * P], ptt[:],
                        attnT8[:, h, 0, qt * P:(qt + 1) * P])


def _emit_stage_b(nc, tc, stages, pools, tensors):
    wBQ, workB, psB, psBq = pools
    (wqb_sb, wkvb_sb, qnT8, cnT8, cosT_sb, sinT_sb,
     kn_sb, vaug, q_nope, qpe_rot, wqbT) = tensors
    KQ = Q_LORA // P
    KKV = KV_LORA // P
    nc.sync.dma_start(wqb_sb[:].rearrange("p k h c -> p (k h c)"), wqbT[:])

    # ---- B-KV first (wkvb preloaded; wqb loads overlap) ----
    # k_nope in [nope, tok] layout; v computed directly transposed
    # ([tok, head*vdim] via cnT-as-stationary) so no DMA transposes.
    for tcks in range(4):
        for ft in range(HL):
            ps = psB.tile([P, 512], F32, tag="up")
            for j in range(KKV // 2):
                for hv, wv in ((0, 0), (0, 1), (1, 0)):
                    nc.tensor.matmul(
                        ps[:], wkvb_sb[:, 2 * j:2 * j + 2, wv, ft * P:(ft + 1) * P],
                        cnT8[:, 2 * j:2 * j + 2, hv, tcks * 512:(tcks + 1) * 512],
                        start=(j == 0 and hv == 0 and wv == 0),
                        stop=(j == KKV // 2 - 1 and hv == 1), perf_mode=DR)
            nc.scalar.mul(kn_sb[:, ft, tcks * 512:(tcks + 1) * 512], ps[:], (1.0 / 512.0))
        for lk in range(4):
            kt = tcks * 4 + lk
            ps = psB.tile([P, 512], F32, tag="up")
            for j in range(KKV // 2):
                for hv, wv in ((0, 0), (0, 1), (1, 0)):
                    nc.tensor.matmul(
                        ps[:], cnT8[:, 2 * j:2 * j + 2, hv, kt * P:(kt + 1) * P],
                        wkvb_sb[:, 2 * j:2 * j + 2, wv, HL * P:HL * P + HL * VHD],
                        start=(j == 0 and hv == 0 and wv == 0),
                        stop=(j == KKV // 2 - 1 and hv == 1), perf_mode=DR)
            nc.scalar.mul(vaug[:, :, kt, 0:VHD],
                          ps[:].rearrange("p (h v) -> p h v", h=HL), (1.0 / 512.0))

    # ---- B-Q: nope (tc-outer) then pe per token tile ----
    for tcks in range(4):
        for ft in range(HL):
            ps = psB.tile([P, 512], F32, tag="up")
            for j in range(KQ // 2):
                for hv, wv in ((0, 0), (0, 1), (1, 0)):
                    nc.tensor.matmul(
                        ps[:], wqb_sb[:, 2 * j:2 * j + 2, wv, ft * P:(ft + 1) * P],
                        qnT8[:, 2 * j:2 * j + 2, hv, tcks * 512:(tcks + 1) * 512],
                        start=(j == 0 and hv == 0 and wv == 0),
                        stop=(j == KQ // 2 - 1 and hv == 1), perf_mode=DR)
            nc.scalar.mul(q_nope[:, ft, tcks * 512:(tcks + 1) * 512], ps[:], (1.0 / 512.0))
        blk = slice(tcks * 512, (tcks + 1) * 512)
        for hp in range(2):
            # q_pe computed directly transposed: [2 heads x 64 pe dims, tok]
            qpe_ps = psBq.tile([P, 512], F32, tag="qpe_ps")
            c0 = HL * P + hp * P
            for j in range(KQ // 2):
                for hv, wv in ((0, 0), (0, 1), (1, 0)):
                    nc.tensor.matmul(
                        qpe_ps[:], wqb_sb[:, 2 * j:2 * j + 2, wv, c0:c0 + P],
                        qnT8[:, 2 * j:2 * j + 2, hv, blk],
                        start=(j == 0 and hv == 0 and wv == 0),
                        stop=(j == KQ // 2 - 1 and hv == 1), perf_mode=DR)
            qtmp = workB.tile([64, 512], BF16, tag="qtmp")
            for hh in range(2):
                h = hp * 2 + hh
                b0 = hh * 64
                nc.vector.tensor_mul(qpe_rot[:, h, blk],
                                     qpe_ps[b0:b0 + 64], cosT_sb[:, blk])
                nc.vector.tensor_mul(qtmp[0:32], qpe_ps[b0 + 32:b0 + 64],
                                     sinT_sb[0:32, blk])
                nc.vector.tensor_mul(qtmp[32:64], qpe_ps[b0:b0 + 32],
                                     sinT_sb[32:64, blk])
                nc.vector.tensor_add(qpe_rot[:, h, blk],
                                     qpe_rot[:, h, blk], qtmp[:])


def _build_nc(stages="ABCD"):
    nc = bacc.Bacc("TRN2", target_bir_lowering=False, debug=False, num_devices=N_CORES)

    hT8 = nc.declare_dram_parameter("hT8", [TT, P, HIDDEN // P * 2 * P], F8, isOutput=False)
    wqaT8 = nc.declare_dram_parameter("wqaT8", [P, HIDDEN // P * 2 * Q_LORA], F8, isOutput=False)
    wkvaT8 = nc.declare_dram_parameter("wkvaT8", [P, HIDDEN // P * 2 * (KV_LORA + ROPE)], F8, isOutput=False)
    cosk_tm = nc.declare_dram_parameter("cosk_tm", [S, ROPE], BF16, isOutput=False)
    sink_tm = nc.declare_dram_parameter("sink_tm", [S, ROPE], BF16, isOutput=False)
    wqbT = nc.declare_dram_parameter("wqbT", [P, Q_LORA // P * 2 * HL * QHD], F8, isOutput=False)
    wkvbT = nc.declare_dram_parameter("wkvbT", [P, KV_LORA // P * 2 * HL * (NOPE + VHD)], F8, isOutput=False)
    woT = nc.declare_dram_parameter("woT", [P, HL * VHD // P * 2 * HIDDEN], F8, isOutput=False)
    cosT_tm = nc.declare_dram_parameter("cosT_tm", [ROPE, S], BF16, isOutput=False)
    sinT_tm = nc.declare_dram_parameter("sinT_tm", [ROPE, S], BF16, isOutput=False)
    masks = nc.declare_dram_parameter("masks", [P, 896], BF16, isOutput=False)
    ident128 = nc.declare_dram_parameter("ident128", [P, P], BF16, isOutput=False)
    outT = nc.declare_dram_parameter("outT", [HIDDEN, S], F32, isOutput=True)

    KH = HIDDEN // P    # 16
    KQ = Q_LORA // P    # 12
    KKV = KV_LORA // P  # 4

    with tile.TileContext(nc) as tc:
        with tc.tile_pool(name="glob", bufs=1) as pp:
            qnT8 = pp.tile([P, KQ, 2, S], F8, tag="qnT8")
            cnT8 = pp.tile([P, KKV, 2, S], F8, tag="cnT8")
            krotT = pp.tile([64, S], BF16, tag="krotT")
            attnT8 = pp.tile([P, HL, 2, S], F8, tag="attnT8")
            cosT_sb = pp.tile([ROPE, S], BF16, tag="cosT")
            sinT_sb = pp.tile([ROPE, S], BF16, tag="sinT")
            nc.sync.dma_start(cosT_sb[:], cosT_tm[:])
            nc.sync.dma_start(sinT_sb[:], sinT_tm[:])
            eps_sb = pp.tile([P, 1], F32, tag="eps")
            wkvb_sb = pp.tile([P, KKV, 2, HL * (NOPE + VHD)], F8, tag="wkvb")
            ident_g = pp.tile([P, P], BF16, tag="ident")
            nc.sync.dma_start(ident_g[:], ident128[:])
            nc.vector.memset(eps_sb[:], EPS)

            # ====== Stage A ==================================================
            if "A" in stages:
                with (
                    tc.tile_pool(name="wA", bufs=1) as wA,
                    tc.tile_pool(name="htp", bufs=4) as htp,
                    tc.tile_pool(name="workA", bufs=2) as workA,
                    tc.tile_pool(name="scr", bufs=2) as scr,
                    tc.tile_pool(name="psA", bufs=2, space="PSUM") as psA,
                    tc.tile_pool(name="psA2", bufs=1, space="PSUM") as psA2,
                    tc.tile_pool(name="psT", bufs=2, space="PSUM") as psT,
                ):
                    # prefetch first two token tiles before the weight stream
                    ht_pre = {}
                    for t in (0, 1, 2, 3):
                        ht_pre[t] = htp.tile([P, KH, 2, P], F8, tag="ht",
                                             name=f"ht{t}")
                        nc.sync.dma_start(
                            ht_pre[t][:].rearrange("p k h t -> p (k h t)"), hT8[t])
                    ident_sb = ident_g
                    wqa_sb = wA.tile([P, KH, 2, Q_LORA], F8, tag="wqa")
                    wkva_sb = wA.tile([P, KH, 2, KV_LORA + ROPE], F8, tag="wkva")
                    costok_k = wA.tile([P, TT, ROPE], BF16, tag="cosk")
                    sintok_k = wA.tile([P, TT, ROPE], BF16, tag="sink")
                    HW2 = KH // 2 * 2 * Q_LORA
                    KW2 = KH // 2 * 2 * (KV_LORA + ROPE)
                    nc.sync.dma_start(
                        wqa_sb[:, :KH // 2].rearrange("p k h c -> p (k h c)"),
                        wqaT8[:, :HW2])
                    nc.sync.dma_start(
                        wkva_sb[:, :KH // 2].rearrange("p k h c -> p (k h c)"),
                        wkvaT8[:, :KW2])
                    nc.sync.dma_start(
                        wqa_sb[:, KH // 2:].rearrange("p k h c -> p (k h c)"),
                        wqaT8[:, HW2:])
                    nc.sync.dma_start(
                        wkva_sb[:, KH // 2:].rearrange("p k h c -> p (k h c)"),
                        wkvaT8[:, KW2:])
                    nc.sync.dma_start(
                        costok_k[:], cosk_tm[:].rearrange("(t p) r -> p t r", p=P))
                    nc.sync.dma_start(
                        sintok_k[:], sink_tm[:].rearrange("(t p) r -> p t r", p=P))
                    nc.sync.dma_start(
                        wkvb_sb[:].rearrange("p k h c -> p (k h c)"), wkvbT[:])

                    for t in range(TT):
                        if t in ht_pre:
                            ht = ht_pre[t]
                        else:
                            ht = htp.tile([P, KH, 2, P], F8, tag="ht", name=f"ht{t}")
                            nc.sync.dma_start(
                                ht[:].rearrange("p k h t -> p (k h t)"), hT8[t])
                        # 3-term compensated fp8 (hi*hi + hi*lo + lo*hi) with
                        # DoubleRow k-tile pairs; each 512-chunk accumulates in
                        # a 1-bank psum, is copied raw to SBUF, and the RMS
                        # norm runs from the SBUF copy to keep psum pressure
                        # at 6 of 8 banks (leaving room for transpose psum).
                        craw = workA.tile([P, KV_LORA + ROPE], BF16, tag="craw")
                        ckv = psA2.tile([P, KV_LORA + ROPE], F32, tag="ckv")
                        for j in range(KH // 2):
                            for hv, wv in ((0, 0), (0, 1), (1, 0)):
                                st = (j == 0 and hv == 0 and wv == 0)
                                sp_ = (j == KH // 2 - 1 and hv == 1)
                                nc.tensor.matmul(
                                    ckv[:, :KV_LORA], ht[:, 2 * j:2 * j + 2, hv],
                                    wkva_sb[:, 2 * j:2 * j + 2, wv, :KV_LORA],
                                    start=st, stop=sp_, perf_mode=DR)
                                nc.tensor.matmul(
                                    ckv[:, KV_LORA:], ht[:, 2 * j:2 * j + 2, hv],
                                    wkva_sb[:, 2 * j:2 * j + 2, wv, KV_LORA:],
                                    start=st, stop=sp_, perf_mode=DR)
                        nc.scalar.copy(craw[:], ckv[:])

                        qraw = workA.tile([P, 3, 512], BF16, tag="qraw")
                        for fc in range(3):
                            qp = psA.tile([P, 512], F32, tag="qp")
                            for j in range(KH // 2):
                                for hv, wv in ((0, 0), (0, 1), (1, 0)):
                                    st = (j == 0 and hv == 0 and wv == 0)
                                    sp_ = (j == KH // 2 - 1 and hv == 1)
                                    nc.tensor.matmul(
                                        qp[:], ht[:, 2 * j:2 * j + 2, hv],
                                        wqa_sb[:, 2 * j:2 * j + 2, wv,
                                               fc * 512:(fc + 1) * 512],
                                        start=st, stop=sp_, perf_mode=DR)
                            nc.scalar.copy(qraw[:, fc], qp[:])

                        sq = scr.tile([P, 512], BF16, tag="sq")
                        parts = scr.tile([P, 4], F32, tag="parts")
                        for fc in range(3):
                            nc.scalar.activation(sq[:], qraw[:, fc], AF.Square,
                                                 accum_out=parts[:, fc:fc + 1])
                        ssum = scr.tile([P, 2], F32, tag="ssum")
                        nc.vector.reduce_sum(ssum[:, 0:1], parts[:, 0:3],
                                             axis=mybir.AxisListType.X)
                        nc.scalar.activation(ssum[:, 1:2], ssum[:, 0:1], AF.Sqrt,
                                             scale=1.0 / Q_LORA, bias=eps_sb[:])
                        rq = scr.tile([P, 1], F32, tag="rq")
                        nc.vector.reciprocal(rq[:], ssum[:, 1:2])
                        qn = workA.tile([P, 3, 512], BF16, tag="qn")
                        for fc in range(3):
                            nc.vector.tensor_scalar_mul(qn[:, fc], qraw[:, fc], rq[:])

                        sq2 = scr.tile([P, 512], BF16, tag="sq2")
                        parts2 = scr.tile([P, 2], F32, tag="parts2")
                        nc.scalar.activation(sq2[:], craw[:, :KV_LORA], AF.Square,
                                             accum_out=parts2[:, 0:1])
                        nc.scalar.activation(parts2[:, 1:2], parts2[:, 0:1], AF.Sqrt,
                                             scale=1.0 / KV_LORA, bias=eps_sb[:])
                        rkv = scr.tile([P, 1], F32, tag="rkv")
                        nc.vector.reciprocal(rkv[:], parts2[:, 1:2])
                        cn = workA.tile([P, 512], BF16, tag="cn")
                        nc.vector.tensor_scalar_mul(cn[:], craw[:, :KV_LORA], rkv[:])

                        kr = workA.tile([P, 2 * ROPE], BF16, tag="kr")
                        tmp = scr.tile([P, ROPE], BF16, tag="tmpr")
                        nc.vector.memset(kr[:, ROPE:], 0.0)
                        nc.vector.tensor_mul(kr[:, 0:ROPE], craw[:, KV_LORA:],
                                             costok_k[:, t])
                        nc.vector.tensor_mul(tmp[:, 0:32], craw[:, KV_LORA + 32:],
                                             sintok_k[:, t, 0:32])
                        nc.vector.tensor_mul(tmp[:, 32:64],
                                             craw[:, KV_LORA:KV_LORA + 32],
                                             sintok_k[:, t, 32:64])
                        nc.vector.tensor_add(kr[:, 0:ROPE], kr[:, 0:ROPE], tmp[:])

                        for f in range(KQ):
                            ptq = psT.tile([P, P], BF16, tag="ptt")
                            nc.tensor.transpose(
                                ptq[:], qn[:, f // 4, (f % 4) * P:((f % 4) + 1) * P],
                                ident_sb[:])
                            nc.vector.tensor_copy(
                                qnT8[:, f, 0, t * P:(t + 1) * P], ptq[:])
                            nc.vector.tensor_sub(
                                qnT8[:, f, 1, t * P:(t + 1) * P], ptq[:],
                                qnT8[:, f, 0, t * P:(t + 1) * P])
                        for f in range(KKV):
                            ptc = psT.tile([P, P], BF16, tag="ptt")
                            nc.tensor.transpose(
                                ptc[:], cn[:, f * P:(f + 1) * P], ident_sb[:])
                            nc.vector.tensor_copy(
                                cnT8[:, f, 0, t * P:(t + 1) * P], ptc[:])
                            nc.vector.tensor_sub(
                                cnT8[:, f, 1, t * P:(t + 1) * P], ptc[:],
                                cnT8[:, f, 0, t * P:(t + 1) * P])
                        ptk = psT.tile([P, P], BF16, tag="ptt")
                        nc.tensor.transpose(ptk[:], kr[:], ident_sb[:])
                        nc.vector.tensor_copy(krotT[:, t * P:(t + 1) * P], ptk[0:64])

            # ====== Stages B + C ============================================
            with tc.tile_pool(name="outs", bufs=1) as outs:
                kn_sb = outs.tile([P, HL, S], BF16, tag="kn_sb")
                vaug = outs.tile([P, HL, KT, VHD + 16], BF16, tag="vaug")
                q_nope = outs.tile([P, HL, S], BF16, tag="q_nope")
                qpe_rot = outs.tile([64, HL, S], BF16, tag="qpe_rot")
                nc.vector.memset(vaug[:, :, :, VHD], 1.0)

                psS = tc.alloc_tile_pool(name="psS", bufs=3, space="PSUM")
                with (
                    tc.tile_pool(name="wBQ", bufs=1) as wBQ,
                    tc.tile_pool(name="workB", bufs=2) as workB,
                    tc.tile_pool(name="psB", bufs=3, space="PSUM") as psB,
                    tc.tile_pool(name="psBq", bufs=2, space="PSUM") as psBq,
                ):
                    wqb_sb = wBQ.tile([P, KQ, 2, HL * QHD], F8, tag="wqb")
                    if "B" in stages:
                        _emit_stage_b(
                            nc, tc, stages, (wBQ, workB, psB, psBq),
                            (wqb_sb, wkvb_sb, qnT8, cnT8, cosT_sb, sinT_sb,
                             kn_sb, vaug, q_nope, qpe_rot, wqbT))

                with (
                    tc.tile_pool(name="wD", bufs=1) as wD,
                    tc.tile_pool(name="workD", bufs=4) as workD,
                ):
                    wo_sb = wD.tile([P, HL, 2, HIDDEN], F8, tag="wo")
                    if "D" in stages:
                        nc.sync.dma_start(
                            wo_sb[:].rearrange("p k h c -> p (k h c)"), woT[:])
                    if "C" in stages:
                        _emit_stage_c(nc, tc, psS, kn_sb, q_nope, qpe_rot, krotT,
                                      vaug, attnT8, masks, ident_g)
                    if "D" in stages:
                        with tc.tile_pool(name="psD", bufs=4, space="PSUM") as psD:
                            for tcks in range(4):
                                for ot in range(HIDDEN // P):
                                    ps = psD.tile([P, 512], F32, tag="wops")
                                    for j in range(HL // 2):
                                        for hv, wv in ((0, 0), (0, 1), (1, 0)):
                                            nc.tensor.matmul(
                                                ps[:],
                                                wo_sb[:, 2 * j:2 * j + 2, wv,
                                                      ot * P:(ot + 1) * P],
                                                attnT8[:, 2 * j:2 * j + 2, hv,
                                                       tcks * 512:(tcks + 1) * 512],
                                                start=(j == 0 and hv == 0 and wv == 0),
                                                stop=(j == HL // 2 - 1 and hv == 1),
                                                perf_mode=DR)
                                    ob = workD.tile([P, 512], F32, tag="ob")
                                    nc.scalar.mul(ob[:], ps[:], (1.0 / 512.0))
                                    nc.sync.dma_start(
                                        outT[ot * P:(ot + 1) * P,
                                             tcks * 512:(tcks + 1) * 512], ob[:])
                psS.release()

    nc.compile()
    return nc


_NC_CACHE = {}
_LAST_RES = None
_LAST_IN_MAPS = None


def _get_nc(stages="ABCD"):
    if stages not in _NC_CACHE:
        _NC_CACHE[stages] = _build_nc(stages)
    return _NC_CACHE[stages]


def kernel(hidden_states, position_ids, wq_a, q_a_ln_w, wq_b, wkv_a, kv_a_ln_w,
           wkv_b, wo):
    hidden_states = np.asarray(hidden_states, dtype=np.float32)
    position_ids = np.asarray(position_ids)
    wq_a = np.asarray(wq_a, dtype=np.float32)
    wq_b = np.asarray(wq_b, dtype=np.float32)
    wkv_a = np.asarray(wkv_a, dtype=np.float32)
    wkv_b = np.asarray(wkv_b, dtype=np.float32)
    wo = np.asarray(wo, dtype=np.float32)
    # fold RMSNorm elementwise weights into the up-projections (exact)
    wq_b = wq_b * np.asarray(q_a_ln_w, dtype=np.float32)[None, :]
    wkv_b = wkv_b * np.asarray(kv_a_ln_w, dtype=np.float32)[None, :]
    assert hidden_states.shape == (B, S, HIDDEN)

    cos_t, sin_t = _yarn_cos_sin(S)

    # --- weight preprocessing (shared across cores in each batch group) ---
    # wq_b rows permuted: per head [nope(128); pe perm64(64)], heads packed as
    # [h0..h3 nope][pe pairs at 64-row offsets]
    wqbT_groups = []
    wkvbT_groups = []
    woT_groups = []
    for g in range(4):
        heads = range(4 * g, 4 * g + 4)
        rows = []
        for h in heads:
            rows.append(np.arange(h * QHD, h * QHD + NOPE))
        pe_rows = []
        for h in heads:
            pe_rows.append(h * QHD + NOPE + _PERM64)
        rows = np.concatenate(rows + pe_rows)
        w8 = _f8pair(wq_b[rows].T, 512.0)                     # [Q_LORA, 2, 768]
        wqbT_groups.append(np.ascontiguousarray(
            w8.reshape(Q_LORA // P, P, 2, -1).transpose(1, 0, 2, 3)
            .reshape(P, -1)))

        rows = []
        for h in heads:
            rows.append(np.arange(h * (NOPE + VHD), h * (NOPE + VHD) + NOPE))
        for h in heads:
            rows.append(np.arange(h * (NOPE + VHD) + NOPE, (h + 1) * (NOPE + VHD)))
        rows = np.concatenate(rows)
        w8 = _f8pair(wkv_b[rows].T, 512.0)                    # [KV_LORA, 2, 1024]
        wkvbT_groups.append(np.ascontiguousarray(
            w8.reshape(KV_LORA // P, P, 2, -1).transpose(1, 0, 2, 3)
            .reshape(P, -1)))

        cols = np.concatenate([np.arange(h * VHD, (h + 1) * VHD) for h in heads])
        w8 = _f8pair(wo[:, cols].T, 512.0)                    # [512, 2, HIDDEN]
        woT_groups.append(np.ascontiguousarray(
            w8.reshape(HL * VHD // P, P, 2, -1).transpose(1, 0, 2, 3)
            .reshape(P, -1)))

    KH = HIDDEN // P
    wqaT8 = np.ascontiguousarray(
        _f8pair(wq_a.T, S_WA).reshape(KH, P, 2, Q_LORA).transpose(1, 0, 2, 3)
        .reshape(P, KH * 2 * Q_LORA))
    wkva_perm = wkv_a.copy()
    wkva_perm[KV_LORA:] = wkv_a[KV_LORA + _PERM64]
    wkvaT8 = np.ascontiguousarray(
        _f8pair(wkva_perm.T, S_WA).reshape(KH, P, 2, KV_LORA + ROPE)
        .transpose(1, 0, 2, 3).reshape(P, KH * 2 * (KV_LORA + ROPE)))

    x_idx = np.arange(896)[None, :]
    p_idx = np.arange(P)[:, None]
    masks = _bf16((x_idx >= 384 + p_idx).astype(np.float32))
    ident128 = _bf16(np.eye(P, dtype=np.float32))

    # --- per-batch rope tables ---
    inv_s = 1.0 / (S_H * S_WA)
    batch_tabs = []
    for beta in range(B):
        pos = position_ids[beta].astype(np.int64)
        cg = cos_t[pos]          # [S, 64]
        sg = sin_t[pos]
        sin_s = np.concatenate([-sg[:, :32], sg[:, 32:]], axis=1)
        h8 = _f8pair(hidden_states[beta].T, S_H)          # [HIDDEN, 2, S]
        h8 = (h8.reshape(KH, P, 2, TT, P).transpose(3, 1, 0, 2, 4)
              .reshape(TT, P, KH * 2 * P))
        batch_tabs.append((
            _bf16(cg.T / 512.0), _bf16(sin_s.T / 512.0),
            _bf16(cg * inv_s), _bf16(sin_s * inv_s),
            np.ascontiguousarray(h8),
        ))

    in_maps = []
    for c in range(N_CORES):
        beta, g = c // 4, c % 4
        cgT, sin_sT, cgk, sink, hT8 = batch_tabs[beta]
        in_maps.append({
            "hT8": hT8,
            "wqaT8": wqaT8,
            "wkvaT8": wkvaT8,
            "wqbT": wqbT_groups[g],
            "wkvbT": wkvbT_groups[g],
            "woT": woT_groups[g],
            "cosT_tm": cgT,
            "sinT_tm": sin_sT,
            "cosk_tm": cgk,
            "sink_tm": sink,
            "masks": masks,
            "ident128": ident128,
        })

    nc = _get_nc()
    global _LAST_RES, _LAST_IN_MAPS
    _LAST_IN_MAPS = in_maps
    res = run_bass_kernel_spmd(nc, in_maps, core_ids=list(range(N_CORES)))
    _LAST_RES = res

    out = np.zeros((B, S, HIDDEN), dtype=np.float32)
    for c in range(N_CORES):
        out[c // 4] += res.results[c]["outT"].T
    return out

